# revision 8
# baseline (speedup 1.0000x reference)
"""Trainium2 Bass kernel for nn_MixtralBinaryDiff (SwiGLU MLP with BitDelta
binary-diff weights), tensor-parallel over 8 NeuronCores.

Math (per reference):
    Wk = mean_wk + ck * sign(wk - mean_wk),  ck = mean|wk - mean_wk|  (global)
    gate = x @ W1^T ; up = x @ W3^T ; h = silu(gate) * up ; out = h @ W2^T

Sharding (Megatron-style on the intermediate dim):
    core r holds rows [r*1792,(r+1)*1792) of w1/w3 (+bases) and the matching
    columns of w2; hidden_states is replicated. Each core computes a full
    [T, HID] partial of the down-projection; a chunked ReduceScatter sums the
    partials and leaves each core with an interleaved token shard, which the
    host reassembles.

All matmuls run in bf16 (fp32 PSUM accumulation). The binarized weights are
materialized on the fly from bf16 temps: S = sign(w-base) and T = bf16(base),
combined as W = T + c*S once the AllReduced coefficient c is known.
"""

import numpy as np

B, S, HID, INTER = 2, 2048, 4096, 14336
NCORES = 8
T = B * S


def build_mlp_nc(ncores, t, hid, inter, tb=256, fake_cc=False):
    """Build the Bass module for one core (SPMD: all cores run the same
    program on different shards). Returns (nc, input_names, out_name)."""
    import concourse.bass as bass
    import concourse.mybir as mybir
    from concourse import bass_isa
    import concourse.tile as tile
    from concourse import bacc

    f32 = mybir.dt.float32
    bf16 = mybir.dt.bfloat16
    Alu = mybir.AluOpType
    Act = mybir.ActivationFunctionType
    Ax = mybir.AxisListType

    iloc = inter // ncores
    KH = hid // 128            # hid k-tiles (32)
    KI = iloc // 128           # local-inter k-tiles (14)
    NTB = t // tb              # token blocks for gate/up (16)
    IT = iloc // 128           # i-tiles in gate/up (14)
    MT = t // 128              # token tiles in down (32)
    MCH = min(8, MT)           # m-tiles per ReduceScatter chunk
    NCHUNK = MT // MCH         # RS chunks (4)
    CH_T = MCH * 128           # tokens per chunk (1024)
    RS_T = CH_T // ncores      # output rows per chunk per core (128)
    NDQ = max(1, hid // 512)   # 512-wide n blocks in down (8)
    NH = 2 if NDQ >= 2 else 1  # psum halves in down
    NQ = NDQ // NH             # 512-blocks per half (4)
    NHW = hid // NH            # free width per half (2048)
    NTOT = float(inter) * float(hid)  # coeff divisor (global count)
    rg = [list(range(ncores))]

    assert hid % 128 == 0 and iloc % 128 == 0 and t % tb == 0 and tb % 128 == 0
    assert MT % MCH == 0 and CH_T % ncores == 0 and RS_T % 1 == 0

    nc = bacc.Bacc(None, target_bir_lowering=False, debug=False,
                   num_devices=ncores)

    x_ext = nc.dram_tensor("hidden_states", [t, hid], f32, kind="ExternalInput")
    w1_ext = nc.dram_tensor("w1", [iloc, hid], f32, kind="ExternalInput")
    mw1_ext = nc.dram_tensor("mean_w1", [iloc, hid], f32, kind="ExternalInput")
    w3_ext = nc.dram_tensor("w3", [iloc, hid], f32, kind="ExternalInput")
    mw3_ext = nc.dram_tensor("mean_w3", [iloc, hid], f32, kind="ExternalInput")
    w2_ext = nc.dram_tensor("w2", [hid, iloc], f32, kind="ExternalInput")
    mw2_ext = nc.dram_tensor("mean_w2", [hid, iloc], f32, kind="ExternalInput")
    out_ext = nc.dram_tensor("out", [t // ncores, hid], f32,
                             kind="ExternalOutput")

    with tile.TileContext(nc) as tc:
        with (
            tc.tile_pool(name="dram", bufs=1, space="DRAM") as dram,
            tc.tile_pool(name="consts", bufs=1) as cpool,
            tc.tile_pool(name="p1", bufs=2) as p1,
            tc.tile_pool(name="p1red", bufs=1) as p1red,
        ):
            # ---- internal DRAM buffers ----
            xbf = dram.tile([t, hid], bf16)
            s1d = dram.tile([iloc, hid], bf16)
            t1d = dram.tile([iloc, hid], bf16)
            s3d = dram.tile([iloc, hid], bf16)
            t3d = dram.tile([iloc, hid], bf16)
            s2d = dram.tile([hid, iloc], bf16)
            t2d = dram.tile([hid, iloc], bf16)
            gd = dram.tile([iloc, t], bf16)   # silu(gate), [i, t] layout
            hd = dram.tile([iloc, t], bf16)   # h = silu(gate)*up, [i, t]
            pout = dram.tile([t, hid], f32)   # down-proj partial
            shared = "Shared" if ncores > 4 else "Local"
            cins = [dram.tile([1, 8], f32, name=f"cin{i}") for i in range(3)]
            couts = [dram.tile([1, 8], f32, name=f"cout{i}",
                               addr_space=shared) for i in range(3)]
            rsout = dram.tile([t // ncores, hid], f32)

            cbrs = [cpool.tile([128, 1], f32, name=f"cbr{i}") for i in range(3)]

            # ---- pass 1 over one weight-pair: temps + |d| reduce + AR ----
            def pass1(w_e, mw_e, s_d, t_d, rows, fdim, idx):
                nt = rows // 128
                fchunk = 1024 if fdim % 1024 == 0 else fdim // 2
                ncf = fdim // fchunk
                red = p1red.tile([128, nt * ncf], f32, name=f"red{idx}")
                for i in range(nt):
                    rs = slice(i * 128, (i + 1) * 128)
                    for j in range(ncf):
                        cs = slice(j * fchunk, (j + 1) * fchunk)
                        wt = p1.tile([128, fchunk], f32, tag="p1w")
                        nc.sync.dma_start(wt[:], w_e[rs, cs])
                        mt = p1.tile([128, fchunk], f32, tag="p1m")
                        nc.sync.dma_start(mt[:], mw_e[rs, cs])
                        db = p1.tile([128, fchunk], bf16, tag="p1d")
                        nc.vector.tensor_tensor(db[:], wt[:], mt[:],
                                                Alu.subtract)
                        sg = p1.tile([128, fchunk], bf16, tag="p1s")
                        nc.scalar.activation(sg[:], db[:], Act.Sign)
                        nc.sync.dma_start(s_d[rs, cs], sg[:])
                        # bf16 base temp via SWDGE cast-on-store
                        nc.gpsimd.dma_start(t_d[rs, cs], mt[:])
                        nc.vector.tensor_reduce(
                            red[:, i * ncf + j:i * ncf + j + 1], db[:],
                            axis=Ax.X, op=Alu.add, apply_absolute_value=True)
                redt = p1red.tile([128, 1], f32, name=f"redt{idx}")
                nc.vector.tensor_reduce(redt[:], red[:], axis=Ax.X, op=Alu.add)
                par = p1red.tile([128, 1], f32, name=f"par{idx}")
                nc.gpsimd.partition_all_reduce(par[:], redt[:], channels=128,
                                               reduce_op=bass_isa.ReduceOp.add)
                cst = cpool.tile([1, 8], f32, name=f"cst{idx}")
                nc.vector.memset(cst[:], 0.0)
                nc.vector.tensor_copy(cst[0:1, 0:1], par[0:1, 0:1])
                nc.sync.dma_start(cins[idx][:], cst[:])
                if fake_cc:
                    nc.sync.dma_start(couts[idx][:], cins[idx][:])
                else:
                    nc.gpsimd.collective_compute(
                        "AllReduce", Alu.add, replica_groups=rg,
                        ins=[cins[idx][:].opt()], outs=[couts[idx][:].opt()])
                cld = cpool.tile([1, 8], f32, name=f"cld{idx}")
                nc.sync.dma_start(cld[:], couts[idx][:])
                csc = cpool.tile([1, 1], f32, name=f"csc{idx}")
                nc.vector.tensor_scalar(csc[:], cld[0:1, 0:1], 1.0 / NTOT,
                                        None, Alu.mult)
                nc.gpsimd.partition_broadcast(cbrs[idx][:], csc[:])

            # w1 first (unblocks gate pass), then xbf casts, then w3, w2.
            pass1(w1_ext, mw1_ext, s1d, t1d, iloc, hid, 0)
            for c in range(NTB):
                ts_ = slice(c * tb, (c + 1) * tb)
                nc.gpsimd.dma_start(xbf[ts_, :], x_ext[ts_, :])
            pass1(w3_ext, mw3_ext, s3d, t3d, iloc, hid, 1)
            pass1(w2_ext, mw2_ext, s2d, t2d, hid, iloc, 2)

            # ---- gate pass then up pass ----
            with (
                tc.tile_pool(name="wres", bufs=1) as wpool,
                tc.tile_pool(name="xt", bufs=2) as xtpool,
                tc.tile_pool(name="stage", bufs=2) as stpool,
                tc.tile_pool(name="evac", bufs=3) as evpool,
                tc.tile_pool(name="ps2", bufs=2, space="PSUM") as ps2,
            ):
                def gateup(s_d, t_d, cbr, is_up):
                    wres = wpool.tile([128, KH, iloc], bf16, tag="wres")
                    for k in range(KH):
                        ks = slice(k * 128, (k + 1) * 128)
                        tt = stpool.tile([128, iloc], bf16, tag="tt")
                        nc.sync.dma_start(tt[:], t_d[:, ks], transpose=True)
                        st = stpool.tile([128, iloc], bf16, tag="st")
                        nc.sync.dma_start(st[:], s_d[:, ks], transpose=True)
                        nc.vector.scalar_tensor_tensor(
                            wres[:, k, :], st[:], cbr[:], tt[:],
                            op0=Alu.mult, op1=Alu.add)
                    for tbi in range(NTB):
                        ts_ = slice(tbi * tb, (tbi + 1) * tb)
                        xt = xtpool.tile([128, KH, tb], bf16, tag="xt")
                        nc.sync.dma_start(xt[:], xbf[ts_, :], transpose=True)
                        for it in range(IT):
                            is_ = slice(it * 128, (it + 1) * 128)
                            pg = ps2.tile([128, tb], f32, tag="pg")
                            for k in range(KH):
                                nc.tensor.matmul(
                                    pg[:], wres[:, k, is_], xt[:, k, :],
                                    start=(k == 0), stop=(k == KH - 1))
                            if not is_up:
                                sig = evpool.tile([128, tb], bf16, tag="sig")
                                nc.scalar.activation(sig[:], pg[:],
                                                     Act.Sigmoid)
                                sg = evpool.tile([128, tb], bf16, tag="sg")
                                nc.vector.tensor_tensor(sg[:], sig[:], pg[:],
                                                        Alu.mult)
                                nc.sync.dma_start(gd[is_, ts_], sg[:])
                            else:
                                sgt = evpool.tile([128, tb], bf16, tag="sgt")
                                nc.sync.dma_start(sgt[:], gd[is_, ts_])
                                ho = evpool.tile([128, tb], bf16, tag="ho")
                                nc.vector.tensor_tensor(ho[:], sgt[:], pg[:],
                                                        Alu.mult)
                                nc.sync.dma_start(hd[is_, ts_], ho[:])

                gateup(s1d, t1d, cbrs[0], is_up=False)
                gateup(s3d, t3d, cbrs[1], is_up=True)

            # ---- down projection + chunked ReduceScatter ----
            with (
                tc.tile_pool(name="w2w", bufs=1) as w2pool,
                tc.tile_pool(name="st2", bufs=2) as st2pool,
                tc.tile_pool(name="hcol", bufs=2) as hcpool,
                tc.tile_pool(name="ot", bufs=3) as otpool,
                tc.tile_pool(name="ps3", bufs=2, space="PSUM") as ps3,
            ):
                w2w = w2pool.tile([128, KI, hid], bf16)
                for g in range(NH):
                    gs = slice(g * NHW, (g + 1) * NHW)
                    for k in range(KI):
                        ks = slice(k * 128, (k + 1) * 128)
                        t2t = st2pool.tile([128, NHW], bf16, tag="t2t")
                        nc.sync.dma_start(t2t[:], t2d[gs, ks], transpose=True)
                        s2t = st2pool.tile([128, NHW], bf16, tag="s2t")
                        nc.sync.dma_start(s2t[:], s2d[gs, ks], transpose=True)
                        nc.vector.scalar_tensor_tensor(
                            w2w[:, k, gs], s2t[:], cbrs[2][:], t2t[:],
                            op0=Alu.mult, op1=Alu.add)
                for c in range(NCHUNK):
                    for g in range(NH):
                        gs = slice(g * NHW, (g + 1) * NHW)
                        for mi in range(MCH):
                            m = c * MCH + mi
                            ms = slice(m * 128, (m + 1) * 128)
                            hcol = hcpool.tile([128, KI, 128], bf16,
                                               tag="hcol")
                            nc.sync.dma_start(
                                hcol[:],
                                hd[:, ms].rearrange("(a p) b -> p a b", p=128))
                            pd = ps3.tile([128, NHW], f32, tag="pd")
                            for k in range(KI):
                                for q in range(NQ):
                                    qs = slice(g * NHW + q * 512,
                                               g * NHW + (q + 1) * 512)
                                    nc.tensor.matmul(
                                        pd[:, q * 512:(q + 1) * 512],
                                        hcol[:, k, :], w2w[:, k, qs],
                                        start=(k == 0), stop=(k == KI - 1))
                            ot = otpool.tile([128, NHW], f32, tag="ot")
                            nc.vector.tensor_copy(ot[:], pd[:])
                            nc.sync.dma_start(pout[ms, gs], ot[:])
                    cts = slice(c * CH_T, (c + 1) * CH_T)
                    crs = slice(c * RS_T, (c + 1) * RS_T)
                    if fake_cc:
                        nc.sync.dma_start(
                            rsout[crs, :],
                            pout[c * CH_T:c * CH_T + RS_T, :])
                    else:
                        nc.gpsimd.collective_compute(
                            "ReduceScatter", Alu.add, replica_groups=rg,
                            ins=[pout[cts, :].opt()], outs=[rsout[crs, :].opt()])
                    nc.sync.dma_start(out_ext[crs, :], rsout[crs, :])

    nc.compile()
    in_names = ["hidden_states", "w1", "mean_w1", "w3", "mean_w3", "w2",
                "mean_w2"]
    return nc, in_names, "out"


_CACHE = {}
LAST_RESULTS = None


def _get_built(key, *args):
    if key not in _CACHE:
        _CACHE[key] = build_mlp_nc(*args)
    return _CACHE[key]


def kernel(hidden_states, w1, mean_w1, w2, mean_w2, w3, mean_w3):
    global LAST_RESULTS
    import os
    # The axon NTFF-profile hook is unavailable in this environment and the
    # trace path would crash on import; force it off.
    os.environ["BASS_NEVER_TRACE"] = "1"
    from concourse import bass_utils

    x = np.ascontiguousarray(np.asarray(hidden_states, dtype=np.float32)
                             .reshape(T, HID))
    w1 = np.asarray(w1, dtype=np.float32)
    mean_w1 = np.asarray(mean_w1, dtype=np.float32)
    w2 = np.asarray(w2, dtype=np.float32)
    mean_w2 = np.asarray(mean_w2, dtype=np.float32)
    w3 = np.asarray(w3, dtype=np.float32)
    mean_w3 = np.asarray(mean_w3, dtype=np.float32)

    nc, in_names, out_name = _get_built("full", NCORES, T, HID, INTER)

    iloc = INTER // NCORES
    in_maps = []
    for r in range(NCORES):
        rs = slice(r * iloc, (r + 1) * iloc)
        in_maps.append({
            "hidden_states": x,
            "w1": np.ascontiguousarray(w1[rs, :]),
            "mean_w1": np.ascontiguousarray(mean_w1[rs, :]),
            "w3": np.ascontiguousarray(w3[rs, :]),
            "mean_w3": np.ascontiguousarray(mean_w3[rs, :]),
            "w2": np.ascontiguousarray(w2[:, rs]),
            "mean_w2": np.ascontiguousarray(mean_w2[:, rs]),
        })

    res = bass_utils.run_bass_kernel_spmd(nc, in_maps,
                                          core_ids=list(range(NCORES)))
    LAST_RESULTS = res

    # Reassemble: chunk c of core r holds tokens [c*1024 + r*128, +128).
    MT = T // 128
    MCH = min(8, MT)
    NCHUNK = MT // MCH
    CH_T = MCH * 128
    RS_T = CH_T // NCORES
    full = np.empty((T, HID), dtype=np.float32)
    for r in range(NCORES):
        o = np.asarray(res.results[r][out_name])
        for c in range(NCHUNK):
            full[c * CH_T + r * RS_T: c * CH_T + (r + 1) * RS_T] = \
                o[c * RS_T:(c + 1) * RS_T]
    return full.reshape(B, S, HID)


# revision 24
# speedup vs baseline: 13.3448x; 13.3448x over previous
"""Trainium2 Bass kernel for nn_MixtralBinaryDiff (SwiGLU MLP with BitDelta
binary-diff weights), tensor-parallel over 8 NeuronCores.

Math (per reference):
    Wk = mean_wk + ck * sign(wk - mean_wk),  ck = mean|wk - mean_wk|  (global)
    gate = x @ W1^T ; up = x @ W3^T ; h = silu(gate) * up ; out = h @ W2^T

Sharding (Megatron-style on the intermediate dim):
    core r holds rows [r*1792,(r+1)*1792) of w1/w3 (+bases) and the matching
    columns of w2; hidden_states is replicated. Each core computes a full
    [T, HID] partial of the down-projection; a chunked ReduceScatter sums the
    partials and leaves each core with an interleaved token shard, which the
    host reassembles.

All matmuls run in bf16 (fp32 PSUM accumulation). The binarized weights are
materialized on the fly from bf16 temps: S = sign(w-base) and T = bf16(base),
combined as W = T + c*S once the AllReduced coefficient c is known.

Emission order doubles as scheduler priority, so each pass-1 reduction phase
is emitted after the compute pass that must start before it: w1-pass (with
the gate-weight transpose loads interleaved), gate matmuls (with the x bf16
casts interleaved), w3-pass, up matmuls, w2-pass, down-proj + ReduceScatter.
"""

import numpy as np

B, S, HID, INTER = 2, 2048, 4096, 14336
NCORES = 8
T = B * S


def build_mlp_nc(ncores, t, hid, inter, tb=256, fake_cc=False, repeat=1):
    """Build the Bass module for one core (SPMD: all cores run the same
    program on different shards). Returns (nc, input_names, out_name)."""
    import concourse.mybir as mybir
    from concourse import bass_isa
    from concourse.bass import _add_dep_helper
    import concourse.tile as tile
    from concourse import bacc

    f32 = mybir.dt.float32
    bf16 = mybir.dt.bfloat16
    Alu = mybir.AluOpType
    Act = mybir.ActivationFunctionType
    Ax = mybir.AxisListType

    iloc = inter // ncores
    KH = hid // 128            # hid k-tiles (32)
    KI = iloc // 128           # local-inter k-tiles (14)
    NTB = t // tb              # token blocks for gate/up (16)
    IT = iloc // 128           # i-tiles in gate/up (14)
    MT = t // 128              # token tiles in down (32)
    MCH = min(8, MT)           # m-tiles per ReduceScatter chunk
    NCHUNK = MT // MCH         # RS chunks (4)
    CH_T = MCH * 128           # tokens per chunk (1024)
    RS_T = CH_T // ncores      # output rows per chunk per core (128)
    NDQ = max(1, hid // 512)   # 512-wide n blocks in down (8)
    NH = 2 if NDQ >= 2 else 1  # psum halves in down
    NQ = NDQ // NH             # 512-blocks per half (4)
    NHW = hid // NH            # free width per half (2048)
    NTOT = float(inter) * float(hid)  # coeff divisor (global count)
    rg = [list(range(ncores))]

    assert hid % 128 == 0 and iloc % 128 == 0 and t % tb == 0 and tb % 128 == 0
    assert MT % MCH == 0 and CH_T % ncores == 0

    nc = bacc.Bacc(None, target_bir_lowering=False, debug=False,
                   num_devices=ncores)

    x_ext = nc.dram_tensor("hidden_states", [t, hid], f32, kind="ExternalInput")
    w1_ext = nc.dram_tensor("w1", [iloc, hid], f32, kind="ExternalInput")
    mw1_ext = nc.dram_tensor("mean_w1", [iloc, hid], f32, kind="ExternalInput")
    w3_ext = nc.dram_tensor("w3", [iloc, hid], f32, kind="ExternalInput")
    mw3_ext = nc.dram_tensor("mean_w3", [iloc, hid], f32, kind="ExternalInput")
    w2_ext = nc.dram_tensor("w2", [hid, iloc], f32, kind="ExternalInput")
    mw2_ext = nc.dram_tensor("mean_w2", [hid, iloc], f32, kind="ExternalInput")
    out_ext = nc.dram_tensor("out", [t // ncores, hid], f32,
                             kind="ExternalOutput")

    def emit_once(tc, dram, cpool, p1, p1red, rep):
        # ---- internal DRAM buffers ----
        # Temps are split per column chunk so a transpose-read of columns
        # becomes ready as soon as its chunk's writes land (tile-granular
        # dependency tracking would otherwise serialize on the whole temp).
        FC1 = 512            # pass-1 column chunk for w1/w3 (fdim=hid)
        FC2 = 256            # pass-1 column chunk for w2 (fdim=iloc)
        FCM = max(FC1, FC2)  # pass-1 staging tile size (shared slots)
        xbf = dram.tile([t, hid], bf16, name=f"xbf{rep}")
        s1ds = [dram.tile([iloc, FC1], bf16, name=f"s1d{rep}_{j}")
                for j in range(hid // FC1)]
        s3ds = [dram.tile([iloc, FC1], bf16, name=f"s3d{rep}_{j}")
                for j in range(hid // FC1)]
        t3ds = [dram.tile([iloc, FC1], bf16, name=f"t3d{rep}_{j}")
                for j in range(hid // FC1)]
        s2ds = [dram.tile([hid, FC2], bf16, name=f"s2d{rep}_{j}")
                for j in range(iloc // FC2)]
        t2ds = [dram.tile([hid, FC2], bf16, name=f"t2d{rep}_{j}")
                for j in range(iloc // FC2)]
        gd = dram.tile([iloc, t], bf16, name=f"gd{rep}")
        hd = dram.tile([iloc, t], bf16, name=f"hd{rep}")
        pout = dram.tile([t, hid], f32, name=f"pout{rep}")
        shared = "Shared" if ncores > 4 else "Local"
        cins = [dram.tile([1, 8], f32, name=f"cin{rep}_{i}") for i in range(3)]
        couts = [dram.tile([1, 8], f32, name=f"cout{rep}_{i}",
                           addr_space=shared) for i in range(3)]
        rsout = dram.tile([t // ncores, hid], f32, name=f"rsout{rep}")

        cbrs = [cpool.tile([128, 1], f32, name=f"cbr{rep}_{i}", tag=f"cbr{i}")
                for i in range(3)]

        # ---- pass-1 over one weight pair: temps + |d| reduce + AllReduce.
        # Walks column chunks in the outer loop; writes sign temps to s_ds
        # chunk tiles; the base temp either goes to t_ds chunk tiles (SWDGE
        # cast) or is handed to t_sink (w1: PE-transposed straight into the
        # resident gate-weight tile while the PE is otherwise idle).
        def pass1(w_e, mw_e, s_ds, t_ds, rows, fdim, fchunk, idx,
                  t_sink=None, pace_after=None):
            nt = rows // 128
            ncf = fdim // fchunk
            red = p1red.tile([128, nt * ncf], f32, name=f"red{rep}_{idx}",
                             tag=f"red{idx}")
            n = -1
            for j in range(ncf):
                for i in range(nt):
                    n += 1
                    rs = slice(i * 128, (i + 1) * 128)
                    cs = slice(j * fchunk, (j + 1) * fchunk)
                    wt = p1.tile([128, fchunk], f32, tag="p1w",
                                 padded_shape=[128, FCM])
                    d1 = nc.sync.dma_start(wt[:], w_e[rs, cs])
                    mt = p1.tile([128, fchunk], f32, tag="p1m",
                                 padded_shape=[128, FCM])
                    d2 = nc.sync.dma_start(mt[:], mw_e[rs, cs])
                    if pace_after is not None:
                        pl = (pace_after if isinstance(pace_after, list)
                              else [pace_after])
                        pa = pl[0] if len(pl) == 1 else pl[min(
                            len(pl) - 1, (n * 4) // (nt * ncf))]
                        _add_dep_helper(d1.ins, pa.ins, sync=False,
                                        reason="phase pacing")
                        _add_dep_helper(d2.ins, pa.ins, sync=False,
                                        reason="phase pacing")
                    db = p1.tile([128, fchunk], bf16, tag="p1d",
                                 padded_shape=[128, FCM])
                    nc.vector.tensor_tensor(db[:], wt[:], mt[:], Alu.subtract)
                    sg = p1.tile([128, fchunk], bf16, tag="p1s",
                                 padded_shape=[128, FCM])
                    nc.scalar.activation(sg[:], db[:], Act.Sign)
                    nc.sync.dma_start(s_ds[j][rs, :], sg[:])
                    if t_sink is not None:
                        t_sink(i, j, fchunk, mt)
                    else:
                        # bf16 base temp via SWDGE cast-on-store
                        nc.gpsimd.dma_start(t_ds[j][rs, :], mt[:])
                    nc.vector.tensor_reduce(
                        red[:, i * ncf + j:i * ncf + j + 1], db[:],
                        axis=Ax.X, op=Alu.add, apply_absolute_value=True)
            redt = p1red.tile([128, 1], f32, name=f"redt{rep}_{idx}",
                              tag=f"redt{idx}")
            gate_inst = nc.vector.tensor_reduce(redt[:], red[:], axis=Ax.X,
                                                op=Alu.add)
            par = p1red.tile([128, 1], f32, name=f"par{rep}_{idx}",
                             tag=f"par{idx}")
            nc.gpsimd.partition_all_reduce(par[:], redt[:], channels=128,
                                           reduce_op=bass_isa.ReduceOp.add)
            cst = cpool.tile([1, 8], f32, name=f"cst{rep}_{idx}",
                             tag=f"cst{idx}")
            nc.vector.memset(cst[:], 0.0)
            nc.vector.tensor_copy(cst[0:1, 0:1], par[0:1, 0:1])
            nc.sync.dma_start(cins[idx][:], cst[:])
            if fake_cc:
                nc.sync.dma_start(couts[idx][:], cins[idx][:])
            else:
                nc.gpsimd.collective_compute(
                    "AllReduce", Alu.add, replica_groups=rg,
                    ins=[cins[idx][:].opt()], outs=[couts[idx][:].opt()])
            cld = cpool.tile([1, 8], f32, name=f"cld{rep}_{idx}",
                             tag=f"cld{idx}")
            nc.sync.dma_start(cld[:], couts[idx][:])
            csc = cpool.tile([1, 1], f32, name=f"csc{rep}_{idx}",
                             tag=f"csc{idx}")
            nc.vector.tensor_scalar(csc[:], cld[0:1, 0:1], 1.0 / NTOT,
                                    None, Alu.mult)
            nc.gpsimd.partition_broadcast(cbrs[idx][:], csc[:])
            return gate_inst

        with (
            tc.tile_pool(name="wres", bufs=1) as wpool,
            tc.tile_pool(name="xt", bufs=2) as xtpool,
            tc.tile_pool(name="stage", bufs=6) as stpool,
            tc.tile_pool(name="xconv", bufs=2) as xcpool,
            tc.tile_pool(name="evac", bufs=2) as evpool,
            tc.tile_pool(name="ps2", bufs=2, space="PSUM") as ps2,
            tc.tile_pool(name="psT", bufs=2, space="PSUM") as psT,
        ):
            wres1 = wpool.tile([128, KH, iloc], bf16, tag="wres",
                               name=f"wres1_{rep}")
            ident = cpool.tile([128, 128], f32, name=f"ident{rep}",
                               tag="ident")
            from concourse.masks import make_identity
            make_identity(nc, ident[:])

            def w1_t_sink(i, j, fchunk, mt):
                nb = fchunk // 128
                tp = psT.tile([128, nb, 128], f32, tag="tp")
                for b in range(nb):
                    nc.tensor.transpose(tp[:, b, :],
                                        mt[:, b * 128:(b + 1) * 128],
                                        ident[:])
                k0 = (j * fchunk) // 128
                nc.vector.tensor_copy(
                    wres1[:, k0:k0 + nb, i * 128:(i + 1) * 128], tp[:])

            g1 = pass1(w1_ext, mw1_ext, s1ds, None, iloc, hid, FC1, 0,
                       t_sink=w1_t_sink)

            # x bf16 conversion through SBUF on the priority-ordered HWDGE
            # path (SWDGE casts would hog the DMA engines from t=0).
            XC = 512
            def convert_x(tbi, pace_after=None):
                for i in range(tbi * tb // 128, (tbi + 1) * tb // 128):
                    rs = slice(i * 128, (i + 1) * 128)
                    for j in range(hid // XC):
                        cs = slice(j * XC, (j + 1) * XC)
                        xl = xcpool.tile([128, XC], f32, tag="xl")
                        d = nc.sync.dma_start(xl[:], x_ext[rs, cs])
                        if pace_after is not None:
                            _add_dep_helper(d.ins, pace_after.ins, sync=False,
                                            reason="phase pacing")
                        xc = xcpool.tile([128, XC], bf16, tag="xc")
                        nc.scalar.activation(xc[:], xl[:], Act.Copy)
                        nc.sync.dma_start(xbf[rs, cs], xc[:])

            convert_x(0)

            # Fold the sign part into wres1 once the coefficient arrives.
            def fold_s(wres, s_ds, cbr):
                for k in range(KH):
                    j, off = (k * 128) // FC1, (k * 128) % FC1
                    st = stpool.tile([128, iloc], bf16, tag="st")
                    nc.sync.dma_start(st[:], s_ds[j][:, off:off + 128],
                                      transpose=True)
                    nc.vector.scalar_tensor_tensor(
                        wres[:, k, :], st[:], cbr[:], wres[:, k, :],
                        op0=Alu.mult, op1=Alu.add)

            fold_s(wres1, s1ds, cbrs[0])

            def gateup(wres, is_up):
                tb_marks = []
                for tbi in range(NTB):
                    ts_ = slice(tbi * tb, (tbi + 1) * tb)
                    if not is_up and tbi + 1 < NTB:
                        convert_x(tbi + 1,
                                  pace_after=tb_marks[-2] if len(tb_marks) > 1
                                  else None)
                    xt = xtpool.tile([128, KH, tb], bf16, tag="xt")
                    nc.sync.dma_start(xt[:], xbf[ts_, :], transpose=True)
                    mm = None
                    for it in range(IT):
                        is_ = slice(it * 128, (it + 1) * 128)
                        pg = ps2.tile([128, tb], f32, tag="pg")
                        for k in range(KH):
                            mm = nc.tensor.matmul(
                                pg[:], wres[:, k, is_], xt[:, k, :],
                                start=(k == 0), stop=(k == KH - 1))
                        if not is_up:
                            sig = evpool.tile([128, tb], bf16, tag="sig")
                            nc.scalar.activation(sig[:], pg[:], Act.Sigmoid)
                            sg = evpool.tile([128, tb], bf16, tag="sg")
                            nc.vector.tensor_tensor(sg[:], sig[:], pg[:],
                                                    Alu.mult)
                            nc.sync.dma_start(gd[is_, ts_], sg[:])
                        else:
                            sgt = evpool.tile([128, tb], bf16, tag="sgt")
                            nc.sync.dma_start(sgt[:], gd[is_, ts_])
                            ho = evpool.tile([128, tb], bf16, tag="ho")
                            nc.vector.tensor_tensor(ho[:], sgt[:], pg[:],
                                                    Alu.mult)
                            nc.sync.dma_start(hd[is_, ts_], ho[:])
                    tb_marks.append(mm)
                return tb_marks

            gate_marks = gateup(wres1, is_up=False)

            g3 = pass1(w3_ext, mw3_ext, s3ds, t3ds, iloc, hid, FC1, 1,
                       pace_after=g1)

            wres3 = wpool.tile([128, KH, iloc], bf16, tag="wres",
                               name=f"wres3_{rep}")
            for k in range(KH):
                j, off = (k * 128) // FC1, (k * 128) % FC1
                nc.sync.dma_start(wres3[:, k, :], t3ds[j][:, off:off + 128],
                                  transpose=True)
            fold_s(wres3, s3ds, cbrs[1])

            gateup(wres3, is_up=True)

            pass1(w2_ext, mw2_ext, s2ds, t2ds, hid, iloc, FC2, 2,
                  pace_after=g3)

        # ---- down projection + chunked ReduceScatter ----
        with (
            tc.tile_pool(name="w2w", bufs=1) as w2pool,
            tc.tile_pool(name="st2", bufs=4) as st2pool,
            tc.tile_pool(name="hcol", bufs=2) as hcpool,
            tc.tile_pool(name="ot", bufs=3) as otpool,
            tc.tile_pool(name="ps3", bufs=2, space="PSUM") as ps3,
        ):
            w2w = w2pool.tile([128, KI, hid], bf16, name=f"w2w{rep}")
            for k in range(KI):
                j, off = (k * 128) // FC2, (k * 128) % FC2
                nc.sync.dma_start(w2w[:, k, :], t2ds[j][:, off:off + 128],
                                  transpose=True)
            for k in range(KI):
                j, off = (k * 128) // FC2, (k * 128) % FC2
                s2t = st2pool.tile([128, hid], bf16, tag="s2t")
                nc.sync.dma_start(s2t[:], s2ds[j][:, off:off + 128],
                                  transpose=True)
                nc.vector.scalar_tensor_tensor(
                    w2w[:, k, :], s2t[:], cbrs[2][:], w2w[:, k, :],
                    op0=Alu.mult, op1=Alu.add)
            for c in range(NCHUNK):
                for g in range(NH):
                    gs = slice(g * NHW, (g + 1) * NHW)
                    for mi in range(MCH):
                        m = c * MCH + mi
                        ms = slice(m * 128, (m + 1) * 128)
                        hcol = hcpool.tile([128, KI, 128], bf16, tag="hcol")
                        nc.sync.dma_start(
                            hcol[:],
                            hd[:, ms].rearrange("(a p) b -> p a b", p=128))
                        pd = ps3.tile([128, NHW], f32, tag="pd")
                        for k in range(KI):
                            for q in range(NQ):
                                qs = slice(g * NHW + q * 512,
                                           g * NHW + (q + 1) * 512)
                                nc.tensor.matmul(
                                    pd[:, q * 512:(q + 1) * 512],
                                    hcol[:, k, :], w2w[:, k, qs],
                                    start=(k == 0), stop=(k == KI - 1))
                        ot = otpool.tile([128, NHW], f32, tag="ot")
                        nc.vector.tensor_copy(ot[:], pd[:])
                        nc.sync.dma_start(pout[ms, gs], ot[:])
                cts = slice(c * CH_T, (c + 1) * CH_T)
                crs = slice(c * RS_T, (c + 1) * RS_T)
                if fake_cc:
                    nc.sync.dma_start(rsout[crs, :],
                                      pout[c * CH_T:c * CH_T + RS_T, :])
                else:
                    nc.gpsimd.collective_compute(
                        "ReduceScatter", Alu.add, replica_groups=rg,
                        ins=[pout[cts, :].opt()], outs=[rsout[crs, :].opt()])
                nc.sync.dma_start(out_ext[crs, :], rsout[crs, :])

    with tile.TileContext(nc) as tc:
        with (
            tc.tile_pool(name="dram", bufs=1, space="DRAM") as dram,
            tc.tile_pool(name="consts", bufs=1) as cpool,
            tc.tile_pool(name="p1", bufs=4) as p1,
            tc.tile_pool(name="p1red", bufs=1) as p1red,
        ):
            for rep in range(repeat):
                emit_once(tc, dram, cpool, p1, p1red, rep)

    nc.compile()
    in_names = ["hidden_states", "w1", "mean_w1", "w3", "mean_w3", "w2",
                "mean_w2"]
    return nc, in_names, "out"


_CACHE = {}
LAST_RESULTS = None


def _get_built(key, *args, **kwargs):
    if key not in _CACHE:
        _CACHE[key] = build_mlp_nc(*args, **kwargs)
    return _CACHE[key]


def kernel(hidden_states, w1, mean_w1, w2, mean_w2, w3, mean_w3):
    global LAST_RESULTS
    import os
    # The axon NTFF-profile hook is unavailable in this environment and the
    # trace path would crash on import; force it off.
    os.environ["BASS_NEVER_TRACE"] = "1"
    from concourse import bass_utils

    x = np.ascontiguousarray(np.asarray(hidden_states, dtype=np.float32)
                             .reshape(T, HID))
    w1 = np.asarray(w1, dtype=np.float32)
    mean_w1 = np.asarray(mean_w1, dtype=np.float32)
    w2 = np.asarray(w2, dtype=np.float32)
    mean_w2 = np.asarray(mean_w2, dtype=np.float32)
    w3 = np.asarray(w3, dtype=np.float32)
    mean_w3 = np.asarray(mean_w3, dtype=np.float32)

    nc, in_names, out_name = _get_built("full", NCORES, T, HID, INTER)

    iloc = INTER // NCORES
    in_maps = []
    for r in range(NCORES):
        rs = slice(r * iloc, (r + 1) * iloc)
        in_maps.append({
            "hidden_states": x,
            "w1": np.ascontiguousarray(w1[rs, :]),
            "mean_w1": np.ascontiguousarray(mean_w1[rs, :]),
            "w3": np.ascontiguousarray(w3[rs, :]),
            "mean_w3": np.ascontiguousarray(mean_w3[rs, :]),
            "w2": np.ascontiguousarray(w2[:, rs]),
            "mean_w2": np.ascontiguousarray(mean_w2[:, rs]),
        })

    res = bass_utils.run_bass_kernel_spmd(nc, in_maps,
                                          core_ids=list(range(NCORES)))
    LAST_RESULTS = res

    # Reassemble: chunk c of core r holds tokens [c*1024 + r*128, +128).
    MT = T // 128
    MCH = min(8, MT)
    NCHUNK = MT // MCH
    CH_T = MCH * 128
    RS_T = CH_T // NCORES
    full = np.empty((T, HID), dtype=np.float32)
    for r in range(NCORES):
        o = np.asarray(res.results[r][out_name])
        for c in range(NCHUNK):
            full[c * CH_T + r * RS_T: c * CH_T + (r + 1) * RS_T] = \
                o[c * RS_T:(c + 1) * RS_T]
    return full.reshape(B, S, HID)


# revision 26
# speedup vs baseline: 15.5843x; 1.1678x over previous
"""Trainium2 Bass kernel for nn_MixtralBinaryDiff (SwiGLU MLP with BitDelta
binary-diff weights), tensor-parallel over 8 NeuronCores.

Math (per reference):
    Wk = mean_wk + ck * sign(wk - mean_wk),  ck = mean|wk - mean_wk|  (global)
    gate = x @ W1^T ; up = x @ W3^T ; h = silu(gate) * up ; out = h @ W2^T

Sharding (Megatron-style on the intermediate dim):
    core r holds rows [r*1792,(r+1)*1792) of w1/w3 (+bases) and the matching
    columns of w2; hidden_states is replicated. Each core computes a full
    [T, HID] partial of the down-projection; a chunked ReduceScatter sums the
    partials and leaves each core with an interleaved token shard, which the
    host reassembles.

All matmuls run in bf16 (fp32 PSUM accumulation). The binarized weights are
materialized on the fly from bf16 temps: S = sign(w-base) and T = bf16(base),
combined as W = T + c*S once the AllReduced coefficient c is known.

Emission order doubles as scheduler priority, so each pass-1 reduction phase
is emitted after the compute pass that must start before it: w1-pass (with
the gate-weight transpose loads interleaved), gate matmuls (with the x bf16
casts interleaved), w3-pass, up matmuls, w2-pass, down-proj + ReduceScatter.
"""

import numpy as np

B, S, HID, INTER = 2, 2048, 4096, 14336
NCORES = 8
T = B * S
RS_MCH = 4                 # token tiles per ReduceScatter chunk


def build_mlp_nc(ncores, t, hid, inter, tb=256, fake_cc=False, repeat=1):
    """Build the Bass module for one core (SPMD: all cores run the same
    program on different shards). Returns (nc, input_names, out_name)."""
    import concourse.mybir as mybir
    from concourse import bass_isa
    from concourse.bass import _add_dep_helper
    import concourse.tile as tile
    from concourse import bacc

    f32 = mybir.dt.float32
    bf16 = mybir.dt.bfloat16
    Alu = mybir.AluOpType
    Act = mybir.ActivationFunctionType
    Ax = mybir.AxisListType

    iloc = inter // ncores
    KH = hid // 128            # hid k-tiles (32)
    KI = iloc // 128           # local-inter k-tiles (14)
    NTB = t // tb              # token blocks for gate/up (16)
    IT = iloc // 128           # i-tiles in gate/up (14)
    MT = t // 128              # token tiles in down (32)
    MCH = min(RS_MCH, MT)      # m-tiles per ReduceScatter chunk
    NCHUNK = MT // MCH         # RS chunks (4)
    CH_T = MCH * 128           # tokens per chunk (1024)
    RS_T = CH_T // ncores      # output rows per chunk per core (128)
    NDQ = max(1, hid // 512)   # 512-wide n blocks in down (8)
    NH = 2 if NDQ >= 2 else 1  # psum halves in down
    NQ = NDQ // NH             # 512-blocks per half (4)
    NHW = hid // NH            # free width per half (2048)
    NTOT = float(inter) * float(hid)  # coeff divisor (global count)
    rg = [list(range(ncores))]

    assert hid % 128 == 0 and iloc % 128 == 0 and t % tb == 0 and tb % 128 == 0
    assert MT % MCH == 0 and CH_T % ncores == 0

    nc = bacc.Bacc(None, target_bir_lowering=False, debug=False,
                   num_devices=ncores)

    x_ext = nc.dram_tensor("hidden_states", [t, hid], f32, kind="ExternalInput")
    w1_ext = nc.dram_tensor("w1", [iloc, hid], f32, kind="ExternalInput")
    mw1_ext = nc.dram_tensor("mean_w1", [iloc, hid], f32, kind="ExternalInput")
    w3_ext = nc.dram_tensor("w3", [iloc, hid], f32, kind="ExternalInput")
    mw3_ext = nc.dram_tensor("mean_w3", [iloc, hid], f32, kind="ExternalInput")
    w2_ext = nc.dram_tensor("w2", [hid, iloc], f32, kind="ExternalInput")
    mw2_ext = nc.dram_tensor("mean_w2", [hid, iloc], f32, kind="ExternalInput")
    out_ext = nc.dram_tensor("out", [t // ncores, hid], f32,
                             kind="ExternalOutput")

    def emit_once(tc, dram, cpool, p1, p1red, rep):
        # ---- internal DRAM buffers ----
        # Temps are split per column chunk so a transpose-read of columns
        # becomes ready as soon as its chunk's writes land (tile-granular
        # dependency tracking would otherwise serialize on the whole temp).
        FC1 = 512            # pass-1 column chunk for w1/w3 (fdim=hid)
        FC2 = 256            # pass-1 column chunk for w2 (fdim=iloc)
        FCM = max(FC1, FC2)  # pass-1 staging tile size (shared slots)
        xbf = dram.tile([t, hid], bf16, name=f"xbf{rep}")
        s1ds = [dram.tile([iloc, FC1], bf16, name=f"s1d{rep}_{j}")
                for j in range(hid // FC1)]
        s3ds = [dram.tile([iloc, FC1], bf16, name=f"s3d{rep}_{j}")
                for j in range(hid // FC1)]
        t3ds = [dram.tile([iloc, FC1], bf16, name=f"t3d{rep}_{j}")
                for j in range(hid // FC1)]
        s2ds = [dram.tile([hid, FC2], bf16, name=f"s2d{rep}_{j}")
                for j in range(iloc // FC2)]
        t2ds = [dram.tile([hid, FC2], bf16, name=f"t2d{rep}_{j}")
                for j in range(iloc // FC2)]
        gd = dram.tile([iloc, t], bf16, name=f"gd{rep}")
        hd = dram.tile([iloc, t], bf16, name=f"hd{rep}")
        pout = dram.tile([t, hid], bf16, name=f"pout{rep}")
        shared = "Shared" if ncores > 4 else "Local"
        cins = [dram.tile([1, 8], f32, name=f"cin{rep}_{i}") for i in range(3)]
        couts = [dram.tile([1, 8], f32, name=f"cout{rep}_{i}",
                           addr_space=shared) for i in range(3)]
        rsout = dram.tile([t // ncores, hid], bf16,
                          name=f"rsout{rep}")

        cbrs = [cpool.tile([128, 1], f32, name=f"cbr{rep}_{i}", tag=f"cbr{i}")
                for i in range(3)]

        # ---- pass-1 over one weight pair: temps + |d| reduce + AllReduce.
        # Walks column chunks in the outer loop; writes sign temps to s_ds
        # chunk tiles; the base temp either goes to t_ds chunk tiles (SWDGE
        # cast) or is handed to t_sink (w1: PE-transposed straight into the
        # resident gate-weight tile while the PE is otherwise idle).
        def pass1(w_e, mw_e, s_ds, t_ds, rows, fdim, fchunk, idx,
                  t_sink=None, pace_after=None):
            nt = rows // 128
            ncf = fdim // fchunk
            red = p1red.tile([128, nt * ncf], f32, name=f"red{rep}_{idx}",
                             tag=f"red{idx}")
            n = -1
            for j in range(ncf):
                for i in range(nt):
                    n += 1
                    rs = slice(i * 128, (i + 1) * 128)
                    cs = slice(j * fchunk, (j + 1) * fchunk)
                    wt = p1.tile([128, fchunk], f32, tag="p1w",
                                 padded_shape=[128, FCM])
                    d1 = nc.sync.dma_start(wt[:], w_e[rs, cs])
                    mt = p1.tile([128, fchunk], f32, tag="p1m",
                                 padded_shape=[128, FCM])
                    d2 = nc.sync.dma_start(mt[:], mw_e[rs, cs])
                    if pace_after is not None:
                        pl = (pace_after if isinstance(pace_after, list)
                              else [pace_after])
                        pa = pl[0] if len(pl) == 1 else pl[min(
                            len(pl) - 1, (n * 4) // (nt * ncf))]
                        _add_dep_helper(d1.ins, pa.ins, sync=False,
                                        reason="phase pacing")
                        _add_dep_helper(d2.ins, pa.ins, sync=False,
                                        reason="phase pacing")
                    db = p1.tile([128, fchunk], bf16, tag="p1d",
                                 padded_shape=[128, FCM])
                    nc.vector.tensor_tensor(db[:], wt[:], mt[:], Alu.subtract)
                    sg = p1.tile([128, fchunk], bf16, tag="p1s",
                                 padded_shape=[128, FCM])
                    nc.scalar.activation(sg[:], db[:], Act.Sign)
                    nc.sync.dma_start(s_ds[j][rs, :], sg[:])
                    if t_sink is not None:
                        t_sink(i, j, fchunk, mt)
                    else:
                        # bf16 base temp via SWDGE cast-on-store
                        nc.gpsimd.dma_start(t_ds[j][rs, :], mt[:])
                    nc.vector.tensor_reduce(
                        red[:, i * ncf + j:i * ncf + j + 1], db[:],
                        axis=Ax.X, op=Alu.add, apply_absolute_value=True)
            redt = p1red.tile([128, 1], f32, name=f"redt{rep}_{idx}",
                              tag=f"redt{idx}")
            gate_inst = nc.vector.tensor_reduce(redt[:], red[:], axis=Ax.X,
                                                op=Alu.add)
            par = p1red.tile([128, 1], f32, name=f"par{rep}_{idx}",
                             tag=f"par{idx}")
            nc.gpsimd.partition_all_reduce(par[:], redt[:], channels=128,
                                           reduce_op=bass_isa.ReduceOp.add)
            cst = cpool.tile([1, 8], f32, name=f"cst{rep}_{idx}",
                             tag=f"cst{idx}")
            nc.vector.memset(cst[:], 0.0)
            nc.vector.tensor_copy(cst[0:1, 0:1], par[0:1, 0:1])
            nc.sync.dma_start(cins[idx][:], cst[:])
            if fake_cc:
                nc.sync.dma_start(couts[idx][:], cins[idx][:])
            else:
                nc.gpsimd.collective_compute(
                    "AllReduce", Alu.add, replica_groups=rg,
                    ins=[cins[idx][:].opt()], outs=[couts[idx][:].opt()])
            cld = cpool.tile([1, 8], f32, name=f"cld{rep}_{idx}",
                             tag=f"cld{idx}")
            nc.sync.dma_start(cld[:], couts[idx][:])
            csc = cpool.tile([1, 1], f32, name=f"csc{rep}_{idx}",
                             tag=f"csc{idx}")
            nc.vector.tensor_scalar(csc[:], cld[0:1, 0:1], 1.0 / NTOT,
                                    None, Alu.mult)
            nc.gpsimd.partition_broadcast(cbrs[idx][:], csc[:])
            return gate_inst

        with (
            tc.tile_pool(name="wres", bufs=1) as wpool,
            tc.tile_pool(name="xt", bufs=2) as xtpool,
            tc.tile_pool(name="stage", bufs=6) as stpool,
            tc.tile_pool(name="xconv", bufs=2) as xcpool,
            tc.tile_pool(name="evac", bufs=2) as evpool,
            tc.tile_pool(name="ps2", bufs=2, space="PSUM") as ps2,
            tc.tile_pool(name="psT", bufs=2, space="PSUM") as psT,
        ):
            wres1 = wpool.tile([128, KH, iloc], bf16, tag="wres",
                               name=f"wres1_{rep}")
            ident = cpool.tile([128, 128], f32, name=f"ident{rep}",
                               tag="ident")
            from concourse.masks import make_identity
            make_identity(nc, ident[:])

            def w1_t_sink(i, j, fchunk, mt):
                nb = fchunk // 128
                tp = psT.tile([128, nb, 128], f32, tag="tp")
                for b in range(nb):
                    nc.tensor.transpose(tp[:, b, :],
                                        mt[:, b * 128:(b + 1) * 128],
                                        ident[:])
                k0 = (j * fchunk) // 128
                nc.vector.tensor_copy(
                    wres1[:, k0:k0 + nb, i * 128:(i + 1) * 128], tp[:])

            g1 = pass1(w1_ext, mw1_ext, s1ds, None, iloc, hid, FC1, 0,
                       t_sink=w1_t_sink)

            # x bf16 conversion through SBUF on the priority-ordered HWDGE
            # path (SWDGE casts would hog the DMA engines from t=0).
            XC = 512
            def convert_x(tbi, pace_after=None):
                for i in range(tbi * tb // 128, (tbi + 1) * tb // 128):
                    rs = slice(i * 128, (i + 1) * 128)
                    for j in range(hid // XC):
                        cs = slice(j * XC, (j + 1) * XC)
                        xl = xcpool.tile([128, XC], f32, tag="xl")
                        d = nc.sync.dma_start(xl[:], x_ext[rs, cs])
                        if pace_after is not None:
                            _add_dep_helper(d.ins, pace_after.ins, sync=False,
                                            reason="phase pacing")
                        xc = xcpool.tile([128, XC], bf16, tag="xc")
                        nc.scalar.activation(xc[:], xl[:], Act.Copy)
                        nc.sync.dma_start(xbf[rs, cs], xc[:])

            convert_x(0)

            # Fold the sign part into wres1 once the coefficient arrives.
            def fold_s(wres, s_ds, cbr):
                for k in range(KH):
                    j, off = (k * 128) // FC1, (k * 128) % FC1
                    st = stpool.tile([128, iloc], bf16, tag="st")
                    nc.sync.dma_start(st[:], s_ds[j][:, off:off + 128],
                                      transpose=True)
                    nc.vector.scalar_tensor_tensor(
                        wres[:, k, :], st[:], cbr[:], wres[:, k, :],
                        op0=Alu.mult, op1=Alu.add)

            fold_s(wres1, s1ds, cbrs[0])

            def gateup(wres, is_up):
                tb_marks = []
                for tbi in range(NTB):
                    ts_ = slice(tbi * tb, (tbi + 1) * tb)
                    if not is_up and tbi + 1 < NTB:
                        convert_x(tbi + 1,
                                  pace_after=tb_marks[-2] if len(tb_marks) > 1
                                  else None)
                    xt = xtpool.tile([128, KH, tb], bf16, tag="xt")
                    nc.sync.dma_start(xt[:], xbf[ts_, :], transpose=True)
                    mm = None
                    for it in range(IT):
                        is_ = slice(it * 128, (it + 1) * 128)
                        pg = ps2.tile([128, tb], f32, tag="pg")
                        for k in range(KH):
                            mm = nc.tensor.matmul(
                                pg[:], wres[:, k, is_], xt[:, k, :],
                                start=(k == 0), stop=(k == KH - 1))
                        if not is_up:
                            sig = evpool.tile([128, tb], bf16, tag="sig")
                            nc.scalar.activation(sig[:], pg[:], Act.Sigmoid)
                            sg = evpool.tile([128, tb], bf16, tag="sg")
                            nc.vector.tensor_tensor(sg[:], sig[:], pg[:],
                                                    Alu.mult)
                            nc.sync.dma_start(gd[is_, ts_], sg[:])
                        else:
                            sgt = evpool.tile([128, tb], bf16, tag="sgt")
                            nc.sync.dma_start(sgt[:], gd[is_, ts_])
                            ho = evpool.tile([128, tb], bf16, tag="ho")
                            nc.vector.tensor_tensor(ho[:], sgt[:], pg[:],
                                                    Alu.mult)
                            nc.sync.dma_start(hd[is_, ts_], ho[:])
                    tb_marks.append(mm)
                return tb_marks

            gate_marks = gateup(wres1, is_up=False)

            g3 = pass1(w3_ext, mw3_ext, s3ds, t3ds, iloc, hid, FC1, 1,
                       pace_after=g1)

            wres3 = wpool.tile([128, KH, iloc], bf16, tag="wres",
                               name=f"wres3_{rep}")
            for k in range(KH):
                j, off = (k * 128) // FC1, (k * 128) % FC1
                nc.sync.dma_start(wres3[:, k, :], t3ds[j][:, off:off + 128],
                                  transpose=True)
            fold_s(wres3, s3ds, cbrs[1])

            gateup(wres3, is_up=True)

            pass1(w2_ext, mw2_ext, s2ds, t2ds, hid, iloc, FC2, 2,
                  pace_after=g3)

        # ---- down projection + chunked ReduceScatter ----
        with (
            tc.tile_pool(name="w2w", bufs=1) as w2pool,
            tc.tile_pool(name="st2", bufs=4) as st2pool,
            tc.tile_pool(name="hcol", bufs=2) as hcpool,
            tc.tile_pool(name="ot", bufs=3) as otpool,
            tc.tile_pool(name="ps3", bufs=2, space="PSUM") as ps3,
        ):
            w2w = w2pool.tile([128, KI, hid], bf16, name=f"w2w{rep}")
            for k in range(KI):
                j, off = (k * 128) // FC2, (k * 128) % FC2
                nc.sync.dma_start(w2w[:, k, :], t2ds[j][:, off:off + 128],
                                  transpose=True)
            for k in range(KI):
                j, off = (k * 128) // FC2, (k * 128) % FC2
                s2t = st2pool.tile([128, hid], bf16, tag="s2t")
                nc.sync.dma_start(s2t[:], s2ds[j][:, off:off + 128],
                                  transpose=True)
                nc.vector.scalar_tensor_tensor(
                    w2w[:, k, :], s2t[:], cbrs[2][:], w2w[:, k, :],
                    op0=Alu.mult, op1=Alu.add)
            for c in range(NCHUNK):
                for g in range(NH):
                    gs = slice(g * NHW, (g + 1) * NHW)
                    for mi in range(MCH):
                        m = c * MCH + mi
                        ms = slice(m * 128, (m + 1) * 128)
                        hcol = hcpool.tile([128, KI, 128], bf16, tag="hcol")
                        nc.sync.dma_start(
                            hcol[:],
                            hd[:, ms].rearrange("(a p) b -> p a b", p=128))
                        pd = ps3.tile([128, NHW], f32, tag="pd")
                        for k in range(KI):
                            for q in range(NQ):
                                qs = slice(g * NHW + q * 512,
                                           g * NHW + (q + 1) * 512)
                                nc.tensor.matmul(
                                    pd[:, q * 512:(q + 1) * 512],
                                    hcol[:, k, :], w2w[:, k, qs],
                                    start=(k == 0), stop=(k == KI - 1))
                        ot = otpool.tile([128, NHW], bf16, tag="ot")
                        nc.vector.tensor_copy(ot[:], pd[:])
                        nc.sync.dma_start(pout[ms, gs], ot[:])
                cts = slice(c * CH_T, (c + 1) * CH_T)
                crs = slice(c * RS_T, (c + 1) * RS_T)
                if fake_cc:
                    nc.sync.dma_start(rsout[crs, :],
                                      pout[c * CH_T:c * CH_T + RS_T, :])
                else:
                    nc.gpsimd.collective_compute(
                        "ReduceScatter", Alu.add, replica_groups=rg,
                        ins=[pout[cts, :].opt()], outs=[rsout[crs, :].opt()])
                # bf16 -> fp32 cast on the way out (SWDGE)
                nc.gpsimd.dma_start(out_ext[crs, :], rsout[crs, :])

    with tile.TileContext(nc) as tc:
        with (
            tc.tile_pool(name="dram", bufs=1, space="DRAM") as dram,
            tc.tile_pool(name="consts", bufs=1) as cpool,
            tc.tile_pool(name="p1", bufs=4) as p1,
            tc.tile_pool(name="p1red", bufs=1) as p1red,
        ):
            for rep in range(repeat):
                emit_once(tc, dram, cpool, p1, p1red, rep)

    nc.compile()
    in_names = ["hidden_states", "w1", "mean_w1", "w3", "mean_w3", "w2",
                "mean_w2"]
    return nc, in_names, "out"


_CACHE = {}
LAST_RESULTS = None


def _get_built(key, *args, **kwargs):
    if key not in _CACHE:
        _CACHE[key] = build_mlp_nc(*args, **kwargs)
    return _CACHE[key]


def kernel(hidden_states, w1, mean_w1, w2, mean_w2, w3, mean_w3):
    global LAST_RESULTS
    import os
    # The axon NTFF-profile hook is unavailable in this environment and the
    # trace path would crash on import; force it off.
    os.environ["BASS_NEVER_TRACE"] = "1"
    from concourse import bass_utils

    x = np.ascontiguousarray(np.asarray(hidden_states, dtype=np.float32)
                             .reshape(T, HID))
    w1 = np.asarray(w1, dtype=np.float32)
    mean_w1 = np.asarray(mean_w1, dtype=np.float32)
    w2 = np.asarray(w2, dtype=np.float32)
    mean_w2 = np.asarray(mean_w2, dtype=np.float32)
    w3 = np.asarray(w3, dtype=np.float32)
    mean_w3 = np.asarray(mean_w3, dtype=np.float32)

    nc, in_names, out_name = _get_built("full", NCORES, T, HID, INTER)

    iloc = INTER // NCORES
    in_maps = []
    for r in range(NCORES):
        rs = slice(r * iloc, (r + 1) * iloc)
        in_maps.append({
            "hidden_states": x,
            "w1": np.ascontiguousarray(w1[rs, :]),
            "mean_w1": np.ascontiguousarray(mean_w1[rs, :]),
            "w3": np.ascontiguousarray(w3[rs, :]),
            "mean_w3": np.ascontiguousarray(mean_w3[rs, :]),
            "w2": np.ascontiguousarray(w2[:, rs]),
            "mean_w2": np.ascontiguousarray(mean_w2[:, rs]),
        })

    res = bass_utils.run_bass_kernel_spmd(nc, in_maps,
                                          core_ids=list(range(NCORES)))
    LAST_RESULTS = res

    # Reassemble: chunk c of core r holds tokens interleaved by chunk/core.
    MT = T // 128
    MCH = min(RS_MCH, MT)
    NCHUNK = MT // MCH
    CH_T = MCH * 128
    RS_T = CH_T // NCORES
    full = np.empty((T, HID), dtype=np.float32)
    for r in range(NCORES):
        o = np.asarray(res.results[r][out_name])
        for c in range(NCHUNK):
            full[c * CH_T + r * RS_T: c * CH_T + (r + 1) * RS_T] = \
                o[c * RS_T:(c + 1) * RS_T]
    return full.reshape(B, S, HID)
